# revision 1
# baseline (speedup 1.0000x reference)
"""Trainium2 Bass kernel for a Mamba block (embed lookup -> residual add ->
RMSNorm -> Mamba(in_proj, causal conv1d, selective scan, out_proj)).

Sharding: tensor-parallel over d_inner across 8 NeuronCores.
- preamble (embed gather + residual + RMSNorm) is token-sharded, then the
  normalized hidden states are AllGathered (bf16, d-major) so every core can
  run its d_inner shard of the Mamba block.
- x_proj partials are AllReduced (contraction over d_inner).
- out_proj partials are ReduceScattered; the host reassembles the full output.

Self-contained: only imports concourse + numpy (+ ml_dtypes for bf16).
"""

import numpy as np
import ml_dtypes

import concourse.bacc as bacc
import concourse.bass as bass
import concourse.mybir as mybir
import concourse.tile as tile
from concourse.bass import AP, IndirectOffsetOnAxis
from concourse.bass_utils import run_bass_kernel_spmd
from concourse.masks import make_identity

F32 = mybir.dt.float32
BF16 = mybir.dt.bfloat16
I32 = mybir.dt.int32
AF = mybir.ActivationFunctionType
ALU = mybir.AluOpType
EPS = 1e-5

BF = ml_dtypes.bfloat16


def _cfg(B, L, DM, DI, NST, DTR, DCONV, V, NC, LC, NRS, rs_f32=True):
    assert (B * L) % NC == 0 and DI % NC == 0
    c = dict(B=B, L=L, DM=DM, DI=DI, NST=NST, DTR=DTR, DCONV=DCONV, V=V,
             NC=NC, LC=LC, NRS=NRS, rs_f32=rs_f32)
    c["DSH"] = DI // NC           # channels per core
    c["TSH"] = (B * L) // NC      # tokens per core (preamble shard)
    c["TT"] = c["TSH"] // 128     # token tiles per core
    c["DT"] = c["DSH"] // 128     # channel tiles per core
    c["KT"] = DM // 128           # d_model k-tiles
    c["MT"] = 2 * c["DSH"] // 128  # xz column tiles
    c["OMT"] = DM // 128          # out_proj m tiles
    c["R2"] = DTR + 2 * NST
    c["NCH"] = B * (L // LC)      # number of scan chunks
    assert c["NCH"] % NRS == 0
    c["TPG"] = (B * L) // NRS     # tokens per reduce-scatter group
    assert c["TSH"] % 128 == 0 and c["DSH"] % 128 == 0 and L % LC == 0
    assert DTR <= 128 and 2 * NST <= 128
    return c


CFG = _cfg(B=2, L=2048, DM=2048, DI=4096, NST=16, DTR=128, DCONV=4, V=50257,
           NC=8, LC=256, NRS=8, rs_f32=True)


def build_nc(c, num_devices=None, reps=1):
    NC = c["NC"]
    B, L, DM, DI = c["B"], c["L"], c["DM"], c["DI"]
    NST, DTR, DCONV, V = c["NST"], c["DTR"], c["DCONV"], c["V"]
    DSH, TSH, TT, DT = c["DSH"], c["TSH"], c["TT"], c["DT"]
    KT, MT, OMT, R2 = c["KT"], c["MT"], c["OMT"], c["R2"]
    LC, NRS, TPG = c["LC"], c["NRS"], c["TPG"]
    BL = B * L
    MSH = DM // NC                # output rows per core after reduce-scatter
    RSDT = F32 if c["rs_f32"] else BF16
    groups = [list(range(NC))]

    nc = bacc.Bacc("TRN2", target_bir_lowering=False, debug=False,
                   num_devices=num_devices or NC)

    # ---- kernel I/O ----
    ids_t = nc.dram_tensor("ids", [128, TT], I32, kind="ExternalInput")
    resid_t = nc.dram_tensor("resid", [TSH, DM], F32, kind="ExternalInput")
    embed_t = nc.dram_tensor("embed", [V, DM], F32, kind="ExternalInput")
    w_in_t = nc.dram_tensor("w_in", [DM, 2 * DSH], BF16, kind="ExternalInput")
    convw_t = nc.dram_tensor("convw", [128, DT * DCONV], F32, kind="ExternalInput")
    convb_t = nc.dram_tensor("convb", [128, DT], F32, kind="ExternalInput")
    xpw_t = nc.dram_tensor("xpw", [DSH, R2], BF16, kind="ExternalInput")
    dtw_t = nc.dram_tensor("dtw", [DTR, DSH], BF16, kind="ExternalInput")
    dtb_t = nc.dram_tensor("dtb", [128, DT], F32, kind="ExternalInput")
    A_t = nc.dram_tensor("A", [128, DT * NST], F32, kind="ExternalInput")
    Dp_t = nc.dram_tensor("Dp", [128, DT], F32, kind="ExternalInput")
    wo_t = nc.dram_tensor("wo", [DSH, DM], BF16, kind="ExternalInput")

    resid_out_t = nc.dram_tensor("resid_out", [TSH, DM], F32, kind="ExternalOutput")
    y_out_t = nc.dram_tensor("y_out", [NRS, MSH, TPG], F32, kind="ExternalOutput")

    with tile.TileContext(nc) as tc:
        with (
            tc.tile_pool(name="dram", bufs=1, space="DRAM") as dram,
            tc.tile_pool(name="const", bufs=1) as const,
        ):
            # ---- constants to SBUF ----
            ids_sb = const.tile([128, TT], I32)
            nc.sync.dma_start(ids_sb[:], ids_t[:])
            convw_sb = const.tile([128, DT * DCONV], F32)
            nc.sync.dma_start(convw_sb[:], convw_t[:])
            convb_sb = const.tile([128, DT], F32)
            nc.sync.dma_start(convb_sb[:], convb_t[:])
            dtb_sb = const.tile([128, DT], F32)
            nc.sync.dma_start(dtb_sb[:], dtb_t[:])
            A_sb = const.tile([128, DT * NST], F32)
            nc.sync.dma_start(A_sb[:], A_t[:])
            Dp_sb = const.tile([128, DT], F32)
            nc.sync.dma_start(Dp_sb[:], Dp_t[:])
            dtw_sb = const.tile([DTR, DSH], BF16)
            nc.sync.dma_start(dtw_sb[:], dtw_t[:])
            xpw_sb = const.tile([128, DT, R2], BF16)
            nc.sync.dma_start(xpw_sb[:], xpw_t[:].rearrange("(k p) r -> p k r", p=128))
            wo_sb = const.tile([128, DT, DM], BF16)
            nc.sync.dma_start(wo_sb[:], wo_t[:].rearrange("(k p) m -> p k m", p=128))
            ident = const.tile([128, 128], BF16)
            make_identity(nc, ident[:])
            zero_b = const.tile([128, 1], F32)
            nc.vector.memset(zero_b[:], 0.0)
            eps_b = const.tile([128, 1], F32)
            nc.vector.memset(eps_b[:], EPS)
            one_b = const.tile([128, 1], F32)
            nc.vector.memset(one_b[:], 1.0)

            for _rep in range(reps):
                # ---- internal DRAM ----
                hs_sh = dram.tile([DM, TSH], BF16, tag=f"hs_sh{_rep}", name=f"hs_sh{_rep}")
                hs_all = dram.tile([NC, DM, TSH], BF16, addr_space="Shared", tag=f"hs_all{_rep}", name=f"hs_all{_rep}")
                xc_hbm = dram.tile([DSH, BL], BF16, tag=f"xc{_rep}", name=f"xc{_rep}")
                x_hbm = dram.tile([DSH, BL], BF16, tag=f"x_hbm{_rep}", name=f"x_hbm{_rep}")
                zs_hbm = dram.tile([DSH, BL], BF16, tag=f"zs_hbm{_rep}", name=f"zs_hbm{_rep}")
                xdbl_par = [dram.tile([R2, L], F32, tag=f"xdp{b}_{_rep}", name=f"xdp{b}_{_rep}")
                            for b in range(B)]
                xdbl = [dram.tile([R2, L], F32, addr_space="Shared", tag=f"xd{b}_{_rep}",
                                  name=f"xd{b}_{_rep}") for b in range(B)]
                op_par = [dram.tile([DM, TPG], RSDT, tag=f"opp{g}_{_rep}", name=f"opp{g}_{_rep}")
                          for g in range(NRS)]
                rs_out = [dram.tile([MSH, TPG], RSDT, tag=f"rso{g}_{_rep}",
                                   name=f"rso{g}_{_rep}") for g in range(NRS)]

                # ================= preamble: gather + residual + rmsnorm ========
                with (
                    tc.tile_pool(name="pre", bufs=3) as pre,
                    tc.tile_pool(name="pre_ps", bufs=2, space="PSUM") as pre_ps,
                ):
                    for j in range(TT):
                        emb = pre.tile([128, DM], F32, tag="emb")
                        nc.gpsimd.indirect_dma_start(
                            out=emb[:], out_offset=None, in_=embed_t[:],
                            in_offset=IndirectOffsetOnAxis(ap=ids_sb[:, j:j + 1], axis=0),
                        )
                        res = pre.tile([128, DM], F32, tag="res")
                        nc.sync.dma_start(res[:], resid_t[j * 128:(j + 1) * 128, :])
                        radd = pre.tile([128, DM], F32, tag="radd")
                        nc.vector.tensor_add(radd[:], emb[:], res[:])
                        nc.sync.dma_start(resid_out_t[j * 128:(j + 1) * 128, :], radd[:])
                        # rms scale = 1/sqrt(mean(x^2) + eps)
                        sq = pre.tile([128, DM], F32, tag="sq")
                        ss = pre.tile([128, 1], F32, tag="ss")
                        nc.scalar.activation(sq[:], radd[:], AF.Square, bias=zero_b[:, 0:1],
                                             accum_out=ss[:])
                        rr = pre.tile([128, 1], F32, tag="rr")
                        nc.scalar.activation(rr[:], ss[:], AF.Sqrt, bias=eps_b[:, 0:1],
                                             scale=1.0 / DM)
                        inv = pre.tile([128, 1], F32, tag="inv")
                        nc.vector.reciprocal(inv[:], rr[:])
                        hsb = pre.tile([128, DM], BF16, tag="hsb")
                        nc.vector.tensor_scalar_mul(hsb[:], radd[:], inv[:, 0:1])
                        # transpose to d-major and store the shard (one DMA)
                        stb = pre.tile([128, KT, 128], BF16, tag="stb")
                        for dcol in range(KT):
                            pt = pre_ps.tile([128, 128], BF16, tag="pt")
                            nc.tensor.transpose(pt[:], hsb[:, dcol * 128:(dcol + 1) * 128],
                                                ident[:])
                            nc.vector.tensor_copy(stb[:, dcol, :], pt[:])
                        nc.gpsimd.dma_start(
                            hs_sh[:, j * 128:(j + 1) * 128]
                            .rearrange("(k p) t -> p k t", p=128), stb[:])

                # ================= allgather hs ================================
                nc.gpsimd.collective_compute(
                    "AllGather", ALU.bypass, replica_groups=groups,
                    ins=[hs_sh[:].opt()], outs=[hs_all[:].opt()],
                )

                # ================= in_proj =====================================
                with (
                    tc.tile_pool(name="wA", bufs=1) as wA,
                    tc.tile_pool(name="hsA", bufs=2) as hsA,
                    tc.tile_pool(name="psA", bufs=4, space="PSUM") as psA,
                    tc.tile_pool(name="xzA", bufs=3) as xzA,
                ):
                    w_sb = wA.tile([128, KT, 2 * DSH], BF16)
                    nc.sync.dma_start(w_sb[:], w_in_t[:].rearrange("(k p) m -> p k m", p=128))
                    for tb in range(NC):
                        hst = hsA.tile([128, KT, TSH], BF16, tag="hst")
                        nc.sync.dma_start(
                            hst[:], hs_all[tb:tb+1].rearrange("o (k p) t -> p (o k) t", p=128))
                        xzb = xzA.tile([128, DT, TSH], BF16, tag="xzb")
                        zsb = xzA.tile([128, DT, TSH], BF16, tag="zsb")
                        for m in range(MT):
                            ps = psA.tile([128, TSH], F32, tag="ps")
                            for k in range(KT):
                                nc.tensor.matmul(
                                    ps[:], lhsT=w_sb[:, k, m * 128:(m + 1) * 128],
                                    rhs=hst[:, k, :], start=(k == 0), stop=(k == KT - 1))
                            if m < DT:
                                nc.scalar.copy(xzb[:, m, :], ps[:])
                            else:
                                sgA = xzA.tile([128, TSH], BF16, tag="sgA")
                                nc.scalar.activation(sgA[:], ps[:], AF.Sigmoid,
                                                     bias=zero_b[:, 0:1])
                                nc.vector.tensor_tensor(zsb[:, m - DT, :], ps[:],
                                                        sgA[:], ALU.mult)
                        nc.gpsimd.dma_start(
                            xc_hbm[:, tb * TSH:(tb + 1) * TSH]
                            .rearrange("(m p) t -> p m t", p=128), xzb[:])
                        nc.gpsimd.dma_start(
                            zs_hbm[:, tb * TSH:(tb + 1) * TSH]
                            .rearrange("(m p) t -> p m t", p=128), zsb[:])

                # ================= causal depthwise conv1d + silu ==============
                with tc.tile_pool(name="cv", bufs=2) as cv:
                    for dti in range(DT):
                        for b in range(B):
                            xcp = cv.tile([128, DCONV - 1 + L], BF16, tag="xcp")
                            nc.vector.memset(xcp[:, 0:DCONV - 1], 0.0)
                            nc.sync.dma_start(
                                xcp[:, DCONV - 1:],
                                xc_hbm[dti * 128:(dti + 1) * 128, b * L:(b + 1) * L])
                            acc = cv.tile([128, L], F32, tag="acc")
                            nc.vector.tensor_scalar(
                                acc[:], xcp[:, 0:L],
                                convw_sb[:, dti * DCONV:dti * DCONV + 1], None, ALU.mult)
                            for jj in range(1, DCONV):
                                nc.vector.scalar_tensor_tensor(
                                    acc[:], xcp[:, jj:jj + L],
                                    convw_sb[:, dti * DCONV + jj:dti * DCONV + jj + 1],
                                    acc[:], ALU.mult, ALU.add)
                            sg = cv.tile([128, L], BF16, tag="sg")
                            nc.scalar.activation(sg[:], acc[:], AF.Sigmoid,
                                                 bias=convb_sb[:, dti:dti + 1])
                            xs = cv.tile([128, L], BF16, tag="xs")
                            nc.vector.scalar_tensor_tensor(
                                xs[:], acc[:], convb_sb[:, dti:dti + 1], sg[:],
                                ALU.add, ALU.mult)
                            nc.sync.dma_start(
                                x_hbm[dti * 128:(dti + 1) * 128, b * L:(b + 1) * L], xs[:])

                # ================= x_proj partials + allreduce =================
                with (
                    tc.tile_pool(name="xp", bufs=2) as xp,
                    tc.tile_pool(name="psX", bufs=2, space="PSUM") as psX,
                    tc.tile_pool(name="xpo", bufs=3) as xpo,
                ):
                    NBT = L // TSH  # token blocks per batch entry
                    for b in range(B):
                        for tb in range(NBT):
                            col0 = b * L + tb * TSH
                            xt = xp.tile([128, DT, TSH], BF16, tag="xt")
                            nc.sync.dma_start(
                                xt[:], x_hbm[:, col0:col0 + TSH]
                                .rearrange("(k p) t -> p k t", p=128))
                            ps1 = psX.tile([DTR, TSH], F32, tag="ps1")
                            ps2 = psX.tile([2 * NST, TSH], F32, tag="ps2")
                            for k in range(DT):
                                nc.tensor.matmul(ps1[:], lhsT=xpw_sb[:, k, 0:DTR],
                                                 rhs=xt[:, k, :], start=(k == 0),
                                                 stop=(k == DT - 1))
                            for k in range(DT):
                                nc.tensor.matmul(ps2[:], lhsT=xpw_sb[:, k, DTR:R2],
                                                 rhs=xt[:, k, :], start=(k == 0),
                                                 stop=(k == DT - 1))
                            s1 = xpo.tile([DTR, TSH], F32, tag="s1")
                            nc.scalar.copy(s1[:], ps1[:])
                            s2 = xpo.tile([2 * NST, TSH], F32, tag="s2")
                            nc.scalar.copy(s2[:], ps2[:])
                            nc.sync.dma_start(
                                xdbl_par[b][0:DTR, tb * TSH:(tb + 1) * TSH], s1[:])
                            nc.sync.dma_start(
                                xdbl_par[b][DTR:R2, tb * TSH:(tb + 1) * TSH], s2[:])
                        nc.gpsimd.collective_compute(
                            "AllReduce", ALU.add, replica_groups=groups,
                            ins=[xdbl_par[b][:].opt()], outs=[xdbl[b][:].opt()],
                        )

                # ================= selective scan + out_proj ===================
                with (
                    tc.tile_pool(name="bc", bufs=1) as bcp,
                    tc.tile_pool(name="scn", bufs=2) as scn,
                    tc.tile_pool(name="scn4", bufs=4) as scn4,
                    tc.tile_pool(name="dap", bufs=1) as dap,
                    tc.tile_pool(name="ht", bufs=2) as htp,
                    tc.tile_pool(name="yp", bufs=DT + 2) as yp,
                    tc.tile_pool(name="psD", bufs=2, space="PSUM") as psD,
                    tc.tile_pool(name="psO", bufs=4, space="PSUM") as psO,
                    tc.tile_pool(name="oev", bufs=1) as oev,
                ):
                    carry_prev = [None] * DT
                    for b in range(B):
                        for ci in range(L // LC):
                            lc0 = ci * LC
                            gcol = b * L + lc0
                            gc = b * (L // LC) + ci       # global chunk index
                            g = (gc * LC) // TPG          # reduce-scatter group
                            gtok = gc * LC - g * TPG      # column offset in group
                            # --- broadcast B and C rows across partitions ---
                            brow = scn.tile([1, NST, LC], F32, tag="brow")
                            nc.sync.dma_start(brow[:], xdbl[b][DTR:DTR + NST, lc0:lc0 + LC])
                            brow16 = scn.tile([1, NST, LC], BF16, tag="brow16")
                            nc.vector.tensor_copy(brow16[:], brow[:])
                            bbc = bcp.tile([128, NST, LC], BF16, tag="bbc")
                            nc.gpsimd.partition_broadcast(bbc[:], brow16[:])
                            crow = scn.tile([1, NST, LC], F32, tag="crow")
                            nc.sync.dma_start(crow[:], xdbl[b][DTR + NST:R2, lc0:lc0 + LC])
                            crow16 = scn.tile([1, NST, LC], BF16, tag="crow16")
                            nc.vector.tensor_copy(crow16[:], crow[:])
                            cbc = bcp.tile([128, NST, LC], BF16, tag="cbc")
                            nc.gpsimd.partition_broadcast(cbc[:], crow16[:])
                            # --- dt_raw chunk (shared across d-tiles) ---
                            dtrf = scn.tile([DTR, LC], F32, tag="dtrf")
                            nc.sync.dma_start(dtrf[:], xdbl[b][0:DTR, lc0:lc0 + LC])
                            dtr16 = scn.tile([DTR, LC], BF16, tag="dtr16")
                            nc.vector.tensor_copy(dtr16[:], dtrf[:])
                            xt4 = scn.tile([128, DT, LC], BF16, tag="xt4")
                            nc.sync.dma_start(
                                xt4[:], x_hbm[:, gcol:gcol + LC]
                                .rearrange("(k p) l -> p k l", p=128))
                            zs4 = scn.tile([128, DT, LC], BF16, tag="zs4")
                            nc.sync.dma_start(
                                zs4[:], zs_hbm[:, gcol:gcol + LC]
                                .rearrange("(k p) l -> p k l", p=128))
                            yf_tiles = []
                            for dti in range(DT):
                                # dt_proj + softplus
                                pdt = psD.tile([128, LC], F32, tag="pdt")
                                nc.tensor.matmul(pdt[:],
                                                 lhsT=dtw_sb[:, dti * 128:(dti + 1) * 128],
                                                 rhs=dtr16[:], start=True, stop=True)
                                dte = scn4.tile([128, LC], F32, tag="dte")
                                nc.scalar.activation(dte[:], pdt[:], AF.Exp,
                                                     bias=dtb_sb[:, dti:dti + 1])
                                dtf = scn4.tile([128, LC], F32, tag="dtf")
                                nc.scalar.activation(dtf[:], dte[:], AF.Ln,
                                                     bias=one_b[:, 0:1])
                                # u = dt * x
                                xt = xt4[:, dti, :]
                                ut = scn4.tile([128, LC], BF16, tag="ut")
                                nc.vector.tensor_tensor(ut[:], dtf[:], xt, ALU.mult)
                                # uB = u (bcast over n) * B_bcast
                                uB = scn.tile([128, NST, LC], BF16, tag="uB")
                                u3 = ut[:].rearrange("p (o l) -> p o l", o=1) \
                                          .to_broadcast([128, NST, LC])
                                nc.vector.tensor_tensor(uB[:], u3, bbc[:], ALU.mult)
                                # dA = exp(dt * A_n)
                                dA = dap.tile([128, NST, LC], BF16, tag="dA")
                                for n in range(NST):
                                    nc.scalar.activation(
                                        dA[:, n, :], dtf[:], AF.Exp,
                                        bias=zero_b[:, 0:1],
                                        scale=A_sb[:, dti * NST + n:dti * NST + n + 1])
                                # hardware linear-recurrence scan along L
                                ht = htp.tile([128, NST, LC], BF16, tag="ht")
                                for n in range(NST):
                                    init = 0.0 if ci == 0 else \
                                        carry_prev[dti][:, n:n + 1]
                                    nc.vector.tensor_tensor_scan(
                                        ht[:, n, :], dA[:, n, :], uB[:, n, :], init,
                                        ALU.mult, ALU.add)
                                # save the last column as next chunk's initial state
                                carry = htp.tile([128, NST], F32, tag=f"carry{dti}")
                                nc.vector.tensor_copy(
                                    carry[:], ht[:, :, LC - 1:LC].rearrange("p n o -> p (n o)"))
                                carry_prev[dti] = carry
                                # y = sum_n ht * C
                                ym = scn.tile([128, NST, LC], BF16, tag="uB")
                                nc.vector.tensor_tensor(ym[:], ht[:], cbc[:], ALU.mult)
                                yr = scn4.tile([128, LC], F32, tag="yr")
                                nc.vector.tensor_reduce(
                                    yr[:], ym[:].rearrange("p n l -> p l n"),
                                    mybir.AxisListType.X, ALU.add)
                                # y += u * D
                                nc.vector.scalar_tensor_tensor(
                                    yr[:], xt, Dp_sb[:, dti:dti + 1], yr[:],
                                    ALU.mult, ALU.add)
                                # y *= silu(z)
                                yf = yp.tile([128, LC], BF16, tag="yf")
                                nc.vector.tensor_tensor(yf[:], yr[:], zs4[:, dti, :],
                                                        ALU.mult)
                                yf_tiles.append(yf)
                            # --- out_proj partial for this chunk (2 DMAs) ---
                            OH = max(1, OMT // 2)
                            for h in range(0, OMT, OH):
                                hn = min(OH, OMT - h)
                                ob = oev.tile([128, OH, LC], RSDT, tag="ob")
                                for mi in range(hn):
                                    m = h + mi
                                    pso = psO.tile([128, LC], F32, tag="pso")
                                    for k in range(DT):
                                        nc.tensor.matmul(
                                            pso[:],
                                            lhsT=wo_sb[:, k, m * 128:(m + 1) * 128],
                                            rhs=yf_tiles[k][:], start=(k == 0),
                                            stop=(k == DT - 1))
                                    nc.scalar.copy(ob[:, mi, :], pso[:])
                                nc.sync.dma_start(
                                    op_par[g][h * 128:(h + hn) * 128, gtok:gtok + LC]
                                    .rearrange("(m p) l -> p m l", p=128),
                                    ob[:, 0:hn, :])
                            # --- reduce-scatter when a group completes ---
                            if (gc + 1) * LC % TPG == 0:
                                nc.gpsimd.collective_compute(
                                    "ReduceScatter", ALU.add, replica_groups=groups,
                                    ins=[op_par[g][:].opt()], outs=[rs_out[g][:].opt()],
                                )
                                if RSDT == F32:
                                    nc.sync.dma_start(
                                        y_out_t[g:g + 1],
                                        rs_out[g][:].rearrange("m t -> (m t)")
                                        .rearrange("(o m t) -> o m t", o=1, m=MSH))
                                else:
                                    with tc.tile_pool(name=f"cvt{g}", bufs=2) as cvt:
                                        for mm0 in range(0, MSH, 128):
                                            mm1 = min(mm0 + 128, MSH)
                                            mp = mm1 - mm0
                                            c16 = cvt.tile([mp, TPG], BF16, tag="c16")
                                            nc.sync.dma_start(
                                                c16[:], rs_out[g][mm0:mm1, :])
                                            c32 = cvt.tile([mp, TPG], F32, tag="c32")
                                            nc.vector.tensor_copy(c32[:], c16[:])
                                            nc.sync.dma_start(
                                                y_out_t[g, mm0:mm1, :], c32[:])
    nc.compile()
    return nc


# ===================== host-side sharding =====================

def make_in_maps(c, inputs):
    NC, DSH, TSH, DT = c["NC"], c["DSH"], c["TSH"], c["DT"]
    B, L, DM, DI = c["B"], c["L"], c["DM"], c["DI"]
    NST, DTR, DCONV, V = c["NST"], c["DTR"], c["DCONV"], c["V"]

    ids = np.asarray(inputs["input_ids"]).reshape(-1).astype(np.int32)
    resid = np.asarray(inputs["residual"], np.float32).reshape(B * L, DM)
    embed = np.ascontiguousarray(np.asarray(inputs["embed"], np.float32))
    norm_w = np.asarray(inputs["norm_w"], np.float32)
    w_in = np.asarray(inputs["in_proj_w"], np.float32) * norm_w[None, :]
    conv_w = np.asarray(inputs["conv_w"], np.float32)
    conv_b = np.asarray(inputs["conv_b"], np.float32)
    xpw = np.asarray(inputs["x_proj_w"], np.float32)
    dtw = np.asarray(inputs["dt_proj_w"], np.float32)
    dtb = np.asarray(inputs["dt_proj_b"], np.float32)
    A = (-np.exp(np.asarray(inputs["A_log"], np.float32))).astype(np.float32)
    Dp = np.asarray(inputs["D_param"], np.float32)
    wo = np.asarray(inputs["out_proj_w"], np.float32)

    in_maps = []
    for cc in range(NC):
        ch = slice(cc * DSH, (cc + 1) * DSH)
        w_x = w_in[cc * DSH:(cc + 1) * DSH, :]
        w_z = w_in[DI + cc * DSH:DI + (cc + 1) * DSH, :]
        w_c = np.concatenate([w_x, w_z], 0).T  # (DM, 2*DSH)
        cw = conv_w[ch].reshape(DT, 128, DCONV).transpose(1, 0, 2).reshape(128, DT * DCONV)
        cb = conv_b[ch].reshape(DT, 128).T
        dtb_c = dtb[ch].reshape(DT, 128).T
        A_c = A[ch].reshape(DT, 128, NST).transpose(1, 0, 2).reshape(128, DT * NST)
        Dp_c = Dp[ch].reshape(DT, 128).T
        in_maps.append({
            "ids": ids[cc * TSH:(cc + 1) * TSH].reshape(-1, 128).T.copy(),
            "resid": resid[cc * TSH:(cc + 1) * TSH].copy(),
            "embed": embed,
            "w_in": np.ascontiguousarray(w_c).astype(BF),
            "convw": np.ascontiguousarray(cw),
            "convb": np.ascontiguousarray(cb),
            "xpw": np.ascontiguousarray(xpw[:, ch].T).astype(BF),
            "dtw": np.ascontiguousarray(dtw[ch, :].T).astype(BF),
            "dtb": np.ascontiguousarray(dtb_c),
            "A": np.ascontiguousarray(A_c),
            "Dp": np.ascontiguousarray(Dp_c),
            "wo": np.ascontiguousarray(wo[:, ch].T).astype(BF),
        })
    return in_maps


def assemble(c, results):
    NC, TSH, DM, B, L = c["NC"], c["TSH"], c["DM"], c["B"], c["L"]
    NRS, TPG, MSH = c["NRS"], c["TPG"], c["DM"] // c["NC"]
    resid = np.concatenate([results[cc]["resid_out"] for cc in range(NC)], 0)
    y = np.stack([results[cc]["y_out"] for cc in range(NC)], 0)  # (NC,NRS,MSH,TPG)
    hs = y.transpose(1, 3, 0, 2).reshape(B * L, DM)
    return (hs.reshape(B, L, DM).astype(np.float32),
            resid.reshape(B, L, DM).astype(np.float32))


_COMPILED = {}


def get_compiled(c=None):
    key = id(c) if c is not None else "default"
    if key not in _COMPILED:
        _COMPILED[key] = build_nc(c or CFG)
    return _COMPILED[key]


def get_compiled_replicated(reps, c=None):
    key = ("rep", reps, id(c) if c is not None else "default")
    if key not in _COMPILED:
        _COMPILED[key] = build_nc(c or CFG, reps=reps)
    return _COMPILED[key], reps


def kernel(**inputs):
    c = CFG
    nc = get_compiled(c)
    in_maps = make_in_maps(c, inputs)
    res = run_bass_kernel_spmd(nc, in_maps, core_ids=list(range(c["NC"])))
    return assemble(c, res.results)

